# revision 14
# baseline (speedup 1.0000x reference)
# Trainium2 Bass kernel for nn_BertAdapter_SLT_49933289783411
#
# Reference computation:
#   y   = tt_linear(x) + bias          (TT-factorized 768->768 linear)
#   out = x + gelu_exact(y)
#
# Key math: the TT cores with ranks [1,5,5,5,5,5,1] factor the 768x768
# weight as W = A @ B with A:(768,5), B:(5,768).  We precompute A,B on
# host (tiny, exact) and run a rank-5 bottleneck matmul on device.
#
# Sharding: data-parallel over the batch dim (8 batch elements -> 8 cores).
# Each core handles x_c:(512,768), pre-transposed on host to x^T (feature-
# major) so the contraction dim lands on SBUF partitions.  Per core:
#   t3    = A^T @ x^T              (5,512)   PSUM accumulate over f-chunks
#   y^T_j = B_j^T @ t3_pad         (128,512) per 128-feature output chunk j
#   o^T_j = x^T_j + gelu_exact(y^T_j + bias_j)
# The host transposes the gathered o^T back.
#
# The whole data path runs in bf16 (x load, mm1, mm2, gelu output,
# residual add, output store); accumulation stays f32 in PSUM.  The
# rel-err budget is 2e-2 and the bf16 path measures ~2.3e-3 end to end,
# while halving both DMA directions and quartering mm1's PE passes
# vs an f32 x path.  The host upcasts the bf16 output to f32.
#
# mm2 runs with K=33 partitions (5 TT ranks + bias row 32): matmul time
# depends only on N, so shrinking K is free, and B then needs only 6
# DMA'd rows instead of a 128-row zero-padded block.
#
# Structure for latency hiding: the 512 batch rows are processed as two
# halves.  Half 0's entire output pipeline (mm2 -> gelu -> residual ->
# store) runs while half 1's x is still streaming from HBM, hiding the
# DMA completion-semaphore latency and overlapping store with load
# traffic.

import numpy as np
import ml_dtypes

import concourse.bass as bass
import concourse.bacc as bacc
import concourse.mybir as mybir
import concourse.tile as tile
from concourse.tile import add_dep_helper
from concourse.bass_utils import run_bass_kernel_spmd

HID = 768
ROWS = 512          # rows per core (one batch element)
HSIZE = (256, 256)
HOFF = (0, 256)
NCORES = 8
FCH = 6             # 768 / 128 feature chunks
RANK = 5
F32 = mybir.dt.float32
BF16 = mybir.dt.bfloat16

N_WARMUP = 16       # dummy PE matmuls: sustained PE power draw trips the
                    # HAM clock un-throttle (~2x matmul rate) after ~3.9us
                    # of gap-free bf16 random-data activity (v5/v6 A/B:
                    # 28 contiguous warmups abutting mm1 tripped it at
                    # +3.9us; 12 warmups with a 0.8us gap before mm1 never
                    # tripped it).  Sized so the warmup chain ends about
                    # when the first x chunks' DMA semaphore arrives.
N_FILLER = 5        # filler matmuls bridging the PE-idle cast-wait gaps
                    # (mm1 -> mm2 handoff) so the power integrator keeps
                    # climbing until the boost trips and doesn't decay
K2 = 33             # mm2 contraction: rows 0..4 = TT rank, row 32 = bias

# packed layout of the input tensor, in bf16 columns:
#   [A (128,30)] [x half0: c0..c5 x 256] [x half1: ...] [B (rows 0:5 + 32)]
A_COLS = FCH * RANK                                # 30
XH_COLS = FCH * HSIZE[0]                           # 1536
BM_OFF = A_COLS + 2 * XH_COLS                      # 3102
XT_COLS = BM_OFF + HID                             # 3870
OUT_COLS = FCH * ROWS                              # 3072

_CACHE = {}


class _LeanTileContext(tile.TileContext):
    """TileContext with a minimal exit sequence.

    The stock exit emits drain + all-engine barrier + per-sem clears +
    barrier (~2-3us).  The runtime re-initializes semaphore state on every
    NEFF execution (verified empirically: repeated executions of the same
    loaded executable stay bit-correct without the clears), so only the
    drain — which makes the kernel end wait for the output DMAs — is kept.
    """

    def _drain_and_barrier(self, tick_clock, wait_clock):
        drain_inst = self.nc.sync.drain()
        # Wait only on the DMA proc clocks (SWDGE+HWDGE, procs 11..26).
        # The stock global-clock wait spans all 27 procs and lowers to a
        # ~50-instruction EVENT_SEMAPHORE chain (~2us of tail).  Output
        # correctness only needs the store DMAs: every compute result the
        # stores depend on is upstream of those DMA sem increments, and
        # each engine's own queue end is awaited by the runtime anyway.
        gc = tick_clock.global_clock
        vals = [gc[p] if p >= 11 else 0 for p in range(27)]
        wait_clock.add_sem_waits(
            drain_inst.ins, tile.ScopedClock({None: tile.VectorClock(vals)})
        )
        popped = self.nc._tile_sem_poison_stack.pop()
        assert popped is self._sem_poison


def _xcol(h, c):
    return A_COLS + h * XH_COLS + c * HSIZE[h]


def _build_program(act=None):
    if act is None:
        act = mybir.ActivationFunctionType.Gelu
    nc = bacc.Bacc(None, target_bir_lowering=False)
    xt = nc.dram_tensor("xt", [128, XT_COLS], BF16, kind="ExternalInput")
    outt = nc.dram_tensor("outt", [128, OUT_COLS], BF16, kind="ExternalOutput")

    with _LeanTileContext(nc) as tc:
        with (
            tc.tile_pool(name="const", bufs=1) as cpool,
            tc.tile_pool(name="xs", bufs=1) as xpool,
            tc.tile_pool(name="work", bufs=4) as wpool,
            tc.tile_pool(name="ps_t3", bufs=1, space="PSUM") as tpool,
            tc.tile_pool(name="ps_o", bufs=4, space="PSUM") as opool,
            tc.tile_pool(name="ps_w", bufs=1, space="PSUM") as wps_pool,
        ):
            # --- PE warmup: matmuls so the HAM clock gate opens.  The
            # trigger looks power-based: the old all-zero warmup data never
            # tripped it (zero operands = no multiplier toggling), while the
            # baseline's real-data fp32 matmuls tripped it ~1.7us in.  Fill
            # with random bits so the warmup actually burns power.  The RNG
            # fill (~1us) runs on Vector, whose other work (B memset, first
            # cast) has slack.
            wsb = cpool.tile([128, 128], BF16)
            nc.vector.random(wsb[:])
            wps = wps_pool.tile([128, 128], F32)

            def warm(n):
                for _ in range(n):
                    nc.tensor.matmul(wps[:], wsb[:], wsb[:], start=True, stop=True)

            warm(N_WARMUP)

            # t3 in bf16; rows 5..31 stay zero, row 32 is all-ones: paired
            # with the bias in B's row 32 it folds the TT bias into mm2
            # (ACT then needs no bias, so gelu can run on j-pairs in one op)
            t3_sb = cpool.tile([128, ROWS], BF16)
            # partition writes must be 32-aligned
            nc.gpsimd.memset(t3_sb[0:32, :], 0.0)
            nc.gpsimd.memset(t3_sb[32:64, :], 1.0)

            x_sb = xpool.tile([128, XT_COLS], BF16)
            a_view = x_sb[:, 0:A_COLS]                    # (128,30)
            bm_view = x_sb[:, BM_OFF:XT_COLS]             # (128,768); rows 0:33 used

            # B rows 5..32 must read as zero for the K=33 mm2
            nc.vector.memset(x_sb[0:32, BM_OFF:XT_COLS], 0.0)

            t3_ps = [
                tpool.tile([RANK, HSIZE[h]], F32, name=f"t3_ps{h}") for h in (0, 1)
            ]

            # x loads: 4 serial DMAs on the Scalar queue (3 chunks each, the
            # first also carries A).  Serial beats parallel queues here: the
            # first chunks' data lands earliest when it has the full HBM
            # bandwidth, and mm1 is chunk-gated.  Scalar (not Sync) because
            # its sequencer reaches 'main' ~0.9us earlier — Sync's preamble
            # has an extra long drain — and the ACT table loads run on the
            # ACT unit concurrently with the DMA issues.
            for d in range(4):
                start = 0 if d == 0 else A_COLS + d * 3 * HSIZE[0]
                end = A_COLS + (d + 1) * 3 * HSIZE[0]
                nc.scalar.dma_start(x_sb[:, start:end], xt[:, start:end])
            # B rows + bias row on the Sync queue (tiny, lands early)
            nc.sync.dma_start(
                x_sb[0:RANK, BM_OFF:XT_COLS], xt[0:RANK, BM_OFF:XT_COLS]
            )
            nc.sync.dma_start(
                x_sb[32:33, BM_OFF:XT_COLS], xt[32:33, BM_OFF:XT_COLS]
            )

            def mm1_half(h, after=None):
                for c in range(FCH):
                    mm = nc.tensor.matmul(
                        t3_ps[h][:],
                        a_view[:, c * RANK : (c + 1) * RANK],
                        x_sb[:, _xcol(h, c) : _xcol(h, c) + HSIZE[h]],
                        start=(c == 0),
                        stop=(c == FCH - 1),
                    )
                    if after is not None:
                        # ordering-only edge: keep these DMA-gated matmuls
                        # out of the strict PE FIFO until half 0's mm2s ran
                        add_dep_helper(
                            mm.ins, after.ins, sync=False,
                            reason="mm1 h1 after phase2 h0 matmuls",
                        )

            def phase2_half(h):
                sz, off = HSIZE[h], HOFF[h]
                nc.vector.tensor_copy(t3_sb[0:RANK, off : off + sz], t3_ps[h][:])
                first_mm = None
                # the whole half's output accumulates into one tile so a
                # single store moves it with 3KB descriptor rows (1KB rows
                # measured ~2.5x slower per byte)
                o_sb = wpool.tile([128, FCH * max(HSIZE)], BF16, tag="o_sb", bufs=2)
                for j0 in range(0, FCH, 2):
                    # two output chunks share one PSUM bank: the first matmul
                    # (start=True) clears the bank's has_written bits, the
                    # second (start=False) overwrites its still-clear region
                    o_ps = opool.tile([128, 2 * max(HSIZE)], F32, tag="o_ps")
                    for k in (0, 1):
                        mm = nc.tensor.matmul(
                            o_ps[:, k * sz : (k + 1) * sz],
                            bm_view[0:K2, (j0 + k) * 128 : (j0 + k + 1) * 128],
                            t3_sb[0:K2, off : off + sz],
                            start=(k == 0),
                            stop=(k == 1),
                        )
                        if first_mm is None:
                            first_mm = mm
                    # one paired gelu halves the per-op ACT overhead on the
                    # critical tail (bias already folded in via mm2)
                    g_sb = wpool.tile([128, 2 * max(HSIZE)], BF16, tag="g_sb", bufs=4)
                    nc.scalar.activation(
                        g_sb[:, : 2 * sz], o_ps[:, : 2 * sz], act, scale=1.0
                    )
                    nc.vector.tensor_add(
                        o_sb[:, j0 * sz : (j0 + 2) * sz],
                        g_sb[:, : 2 * sz],
                        x_sb[:, _xcol(h, j0) : _xcol(h, j0) + 2 * sz],
                    )
                if h == 0:
                    # mid-kernel: one big store, 3KB descriptor rows
                    nc.gpsimd.dma_start(
                        outt[:, 0 : FCH * sz], o_sb[:, : FCH * sz]
                    )
                else:
                    # tail: split so the last store (gating the drain) is
                    # small and its data+completion latency short
                    nc.sync.dma_start(
                        outt[:, FCH * sz : FCH * sz + 4 * sz], o_sb[:, : 4 * sz]
                    )
                    nc.gpsimd.dma_start(
                        outt[:, FCH * sz + 4 * sz : 2 * FCH * sz],
                        o_sb[:, 4 * sz : FCH * sz],
                    )
                return first_mm

            # h1's PE work is emitted after phase2(0) so the strict PE FIFO
            # doesn't block half 0's output pipeline on half 1's loads.
            # Fillers sit in the PE FIFO where the engine would otherwise
            # idle waiting on the casts.
            mm1_half(0)
            warm(N_FILLER)
            first_mm_h0 = phase2_half(0)
            mm1_half(1, after=first_mm_h0)
            warm(N_FILLER)
            phase2_half(1)

    nc.finalize()
    return nc


def _get_program():
    if "nc" not in _CACHE:
        _CACHE["nc"] = _build_program()
    return _CACHE["nc"]


def _host_prep(hidden_states, bias, cores):
    """Collapse TT cores to rank-5 factors; pack consts + x^T per core."""
    c0, c1, c2, c3, c4, c5 = [c.astype(np.float64) for c in cores]
    A = np.einsum("iv,vjw,wkx->ijkx", c0[0], c1, c2).reshape(HID, RANK)
    Bm = np.einsum("xpy,yqz,zr->xpqr", c3, c4, c5[:, :, 0]).reshape(RANK, HID)

    a_p = np.ascontiguousarray(
        A.reshape(FCH, 128, RANK).transpose(1, 0, 2).reshape(128, FCH * RANK)
    ).astype(ml_dtypes.bfloat16)                   # (128, 30)

    xts = []
    for c in range(NCORES):
        xc = hidden_states[c]  # (512, 768)
        xct = xc.T.astype(ml_dtypes.bfloat16)  # (768, 512)
        # per half: [p, c*sz + m~] = x^T[c*128+p, off+m~]
        blocks = [a_p]
        for h in (0, 1):
            sz, off = HSIZE[h], HOFF[h]
            blocks.append(
                xct[:, off : off + sz]
                .reshape(FCH, 128, sz)
                .transpose(1, 0, 2)
                .reshape(128, FCH * sz)
            )
        bm_blk = np.zeros((128, HID), dtype=ml_dtypes.bfloat16)
        bm_blk[:RANK] = Bm.astype(ml_dtypes.bfloat16)
        bm_blk[32] = bias.astype(ml_dtypes.bfloat16)
        blocks.append(bm_blk)
        xts.append(np.ascontiguousarray(np.concatenate(blocks, axis=1)))
    return xts


def _unpack_out(outt_list):
    """outt[p, h*1536 + j*256 + m~] = out[h*256+m~, j*128+p] -> (8,512,768)."""
    outs = []
    for outt in outt_list:
        o = (
            outt.reshape(128, 2, FCH, HSIZE[0])
            .transpose(1, 3, 2, 0)
            .reshape(ROWS, HID)
        )
        outs.append(o)
    return np.stack(outs, axis=0).astype(np.float32)


def run(inputs, trace=False, **spmd_kwargs):
    hidden_states = np.asarray(inputs["hidden_states"], dtype=np.float32)
    bias = np.asarray(inputs["bias"], dtype=np.float32)
    cores = [np.asarray(inputs[f"core{i}"], dtype=np.float32) for i in range(6)]

    xts = _host_prep(hidden_states, bias, cores)
    nc = _get_program()
    in_maps = [{"xt": xts[c]} for c in range(NCORES)]
    res = run_bass_kernel_spmd(
        nc, in_maps, core_ids=list(range(NCORES)), trace=trace, **spmd_kwargs
    )
    out = _unpack_out([res.results[c]["outt"] for c in range(NCORES)])
    if trace:
        return out, res
    return out


def kernel(**inputs):
    return run(inputs)


# revision 17
# speedup vs baseline: 1.0246x; 1.0246x over previous
# Trainium2 Bass kernel for nn_BertAdapter_SLT_49933289783411
#
# Reference computation:
#   y   = tt_linear(x) + bias          (TT-factorized 768->768 linear)
#   out = x + gelu_exact(y)
#
# Key math: the TT cores with ranks [1,5,5,5,5,5,1] factor the 768x768
# weight as W = A @ B with A:(768,5), B:(5,768).  We precompute A,B on
# host (tiny, exact) and run a rank-5 bottleneck matmul on device.
#
# Sharding: data-parallel over the batch dim (8 batch elements -> 8 cores).
# Each core handles x_c:(512,768), pre-transposed on host to x^T (feature-
# major) so the contraction dim lands on SBUF partitions.  Per core:
#   t3    = A^T @ x^T              (5,512)   PSUM accumulate over f-chunks
#   y^T_j = B_j^T @ t3_pad         (128,512) per 128-feature output chunk j
#   o^T_j = x^T_j + gelu_exact(y^T_j + bias_j)
# The host transposes the gathered o^T back.
#
# The whole data path runs in bf16 (x load, mm1, mm2, gelu output,
# residual add, output store); accumulation stays f32 in PSUM.  The
# rel-err budget is 2e-2 and the bf16 path measures ~2.3e-3 end to end,
# while halving both DMA directions and quartering mm1's PE passes
# vs an f32 x path.  The host upcasts the bf16 output to f32.
#
# mm2 runs with K=33 partitions (5 TT ranks + bias row 32): matmul time
# depends only on N, so shrinking K is free, and B then needs only 6
# DMA'd rows instead of a 128-row zero-padded block.
#
# Structure for latency hiding: the 512 batch rows are processed as two
# halves.  Half 0's entire output pipeline (mm2 -> gelu -> residual ->
# store) runs while half 1's x is still streaming from HBM, hiding the
# DMA completion-semaphore latency and overlapping store with load
# traffic.

import numpy as np
import ml_dtypes

import concourse.bass as bass
import concourse.bacc as bacc
import concourse.mybir as mybir
import concourse.tile as tile
from concourse.tile import add_dep_helper
from concourse.bass_utils import run_bass_kernel_spmd

HID = 768
ROWS = 512          # rows per core (one batch element)
HSIZE = (256, 256)
HOFF = (0, 256)
NCORES = 8
FCH = 6             # 768 / 128 feature chunks
RANK = 5
F32 = mybir.dt.float32
BF16 = mybir.dt.bfloat16

N_WARMUP = 42       # dummy PE matmuls: sustained PE power draw trips the
                    # HAM clock un-throttle (~2x matmul rate) after ~3.9us
                    # of gap-free bf16 random-data activity (v5/v6 A/B:
                    # 28 contiguous warmups abutting mm1 tripped it at
                    # +3.9us; 12 warmups with a 0.8us gap before mm1 never
                    # tripped it).  Emitted BEFORE the TileContext so the
                    # PE starts as soon as its preamble ends (~6us) instead
                    # of after the tile entry barrier (~7.6us); sized to
                    # keep the FIFO busy until the first x chunks' DMA
                    # semaphore arrives (~10.3us, jitters +-0.7us).
N_FILLER = 5        # filler matmuls bridging the PE-idle cast-wait gaps
                    # (mm1 -> mm2 handoff) so the power integrator keeps
                    # climbing until the boost trips and doesn't decay
K2 = 33             # mm2 contraction: rows 0..4 = TT rank, row 32 = bias

# packed layout of the input tensor, in bf16 columns:
#   [A (128,30)] [x half0: c0..c5 x 256] [x half1: ...] [B (rows 0:5 + 32)]
A_COLS = FCH * RANK                                # 30
XH_COLS = FCH * HSIZE[0]                           # 1536
BM_OFF = A_COLS + 2 * XH_COLS                      # 3102
XT_COLS = BM_OFF + HID                             # 3870
OUT_COLS = FCH * ROWS                              # 3072

_CACHE = {}


class _LeanTileContext(tile.TileContext):
    """TileContext with a minimal exit sequence.

    The stock exit emits drain + all-engine barrier + per-sem clears +
    barrier (~2-3us).  The runtime re-initializes semaphore state on every
    NEFF execution (verified empirically: repeated executions of the same
    loaded executable stay bit-correct without the clears), so only the
    drain — which makes the kernel end wait for the output DMAs — is kept.
    """

    def _drain_and_barrier(self, tick_clock, wait_clock):
        drain_inst = self.nc.sync.drain()
        # Wait only on the DMA proc clocks (SWDGE+HWDGE, procs 11..26).
        # The stock global-clock wait spans all 27 procs and lowers to a
        # ~50-instruction EVENT_SEMAPHORE chain (~2us of tail).  Output
        # correctness only needs the store DMAs: every compute result the
        # stores depend on is upstream of those DMA sem increments, and
        # each engine's own queue end is awaited by the runtime anyway.
        gc = tick_clock.global_clock
        vals = [gc[p] if p >= 11 else 0 for p in range(27)]
        wait_clock.add_sem_waits(
            drain_inst.ins, tile.ScopedClock({None: tile.VectorClock(vals)})
        )
        popped = self.nc._tile_sem_poison_stack.pop()
        assert popped is self._sem_poison


def _xcol(h, c):
    return A_COLS + h * XH_COLS + c * HSIZE[h]


def _build_program(act=None):
    if act is None:
        act = mybir.ActivationFunctionType.Gelu
    nc = bacc.Bacc(None, target_bir_lowering=False)
    xt = nc.dram_tensor("xt", [128, XT_COLS], BF16, kind="ExternalInput")
    outt = nc.dram_tensor("outt", [128, OUT_COLS], BF16, kind="ExternalOutput")

    # --- pre-tile PE warmup: emitted before the TileContext so it is not
    # held back by the tile entry barrier.  Deliberately race-y: the
    # matmuls read wsb while the RNG fill may still be writing it — the
    # values are irrelevant (wps is never read), the matmuls only need
    # bit-toggling operands to draw power so the HAM clock gate opens.
    # The SBUF/PSUM space frees back to the tile pools below; that reuse
    # is safe because the PE FIFO executes all warmups before any tile
    # matmul, and only the PE touches PSUM.
    with (
        nc.sbuf_tensor("warm_sb", [128, 128], BF16) as wsb,
        nc.psum_tensor("warm_ps", [128, 128], F32) as wps,
    ):
        nc.gpsimd.random(wsb[:])
        for _ in range(N_WARMUP):
            nc.tensor.matmul(wps[:], wsb[:], wsb[:], start=True, stop=True)

    with _LeanTileContext(nc) as tc:
        with (
            tc.tile_pool(name="const", bufs=1) as cpool,
            tc.tile_pool(name="xs", bufs=1) as xpool,
            tc.tile_pool(name="work", bufs=4) as wpool,
            tc.tile_pool(name="ps_t3", bufs=1, space="PSUM") as tpool,
            tc.tile_pool(name="ps_o", bufs=4, space="PSUM") as opool,
            tc.tile_pool(name="ps_w", bufs=1, space="PSUM") as wps_pool,
        ):
            # filler-matmul scratch (random data, same power rationale)
            fsb = cpool.tile([128, 128], BF16)
            nc.vector.random(fsb[:])
            fps = wps_pool.tile([128, 128], F32)

            def warm(n):
                for _ in range(n):
                    nc.tensor.matmul(fps[:], fsb[:], fsb[:], start=True, stop=True)

            # t3 in bf16; rows 5..31 stay zero, row 32 is all-ones: paired
            # with the bias in B's row 32 it folds the TT bias into mm2
            # (ACT then needs no bias, so gelu can run on j-pairs in one op)
            t3_sb = cpool.tile([128, ROWS], BF16)
            # partition writes must be 32-aligned
            nc.gpsimd.memset(t3_sb[0:32, :], 0.0)
            nc.gpsimd.memset(t3_sb[32:64, :], 1.0)

            x_sb = xpool.tile([128, XT_COLS], BF16)
            a_view = x_sb[:, 0:A_COLS]                    # (128,30)
            bm_view = x_sb[:, BM_OFF:XT_COLS]             # (128,768); rows 0:33 used

            # B rows 5..32 must read as zero for the K=33 mm2
            nc.vector.memset(x_sb[0:32, BM_OFF:XT_COLS], 0.0)

            t3_ps = [
                tpool.tile([RANK, HSIZE[h]], F32, name=f"t3_ps{h}") for h in (0, 1)
            ]

            # x loads: 4 serial DMAs on the Scalar queue (3 chunks each, the
            # first also carries A).  Serial beats parallel queues here: the
            # first chunks' data lands earliest when it has the full HBM
            # bandwidth, and mm1 is chunk-gated.  Scalar (not Sync) because
            # its sequencer reaches 'main' ~0.9us earlier — Sync's preamble
            # has an extra long drain — and the ACT table loads run on the
            # ACT unit concurrently with the DMA issues.
            for d in range(4):
                start = 0 if d == 0 else A_COLS + d * 3 * HSIZE[0]
                end = A_COLS + (d + 1) * 3 * HSIZE[0]
                nc.scalar.dma_start(x_sb[:, start:end], xt[:, start:end])
            # B rows + bias row on the Sync queue (tiny, lands early)
            nc.sync.dma_start(
                x_sb[0:RANK, BM_OFF:XT_COLS], xt[0:RANK, BM_OFF:XT_COLS]
            )
            nc.sync.dma_start(
                x_sb[32:33, BM_OFF:XT_COLS], xt[32:33, BM_OFF:XT_COLS]
            )

            def mm1_half(h, after=None):
                for c in range(FCH):
                    mm = nc.tensor.matmul(
                        t3_ps[h][:],
                        a_view[:, c * RANK : (c + 1) * RANK],
                        x_sb[:, _xcol(h, c) : _xcol(h, c) + HSIZE[h]],
                        start=(c == 0),
                        stop=(c == FCH - 1),
                    )
                    if after is not None:
                        # ordering-only edge: keep these DMA-gated matmuls
                        # out of the strict PE FIFO until half 0's mm2s ran
                        add_dep_helper(
                            mm.ins, after.ins, sync=False,
                            reason="mm1 h1 after phase2 h0 matmuls",
                        )

            def phase2_half(h):
                sz, off = HSIZE[h], HOFF[h]
                nc.vector.tensor_copy(t3_sb[0:RANK, off : off + sz], t3_ps[h][:])
                first_mm = None
                # the whole half's output accumulates into one tile so a
                # single store moves it with 3KB descriptor rows (1KB rows
                # measured ~2.5x slower per byte)
                o_sb = wpool.tile([128, FCH * max(HSIZE)], BF16, tag="o_sb", bufs=2)
                for j0 in range(0, FCH, 2):
                    # two output chunks share one PSUM bank: the first matmul
                    # (start=True) clears the bank's has_written bits, the
                    # second (start=False) overwrites its still-clear region
                    o_ps = opool.tile([128, 2 * max(HSIZE)], F32, tag="o_ps")
                    for k in (0, 1):
                        mm = nc.tensor.matmul(
                            o_ps[:, k * sz : (k + 1) * sz],
                            bm_view[0:K2, (j0 + k) * 128 : (j0 + k + 1) * 128],
                            t3_sb[0:K2, off : off + sz],
                            start=(k == 0),
                            stop=(k == 1),
                        )
                        if first_mm is None:
                            first_mm = mm
                    # one paired gelu halves the per-op ACT overhead on the
                    # critical tail (bias already folded in via mm2)
                    g_sb = wpool.tile([128, 2 * max(HSIZE)], BF16, tag="g_sb", bufs=4)
                    nc.scalar.activation(
                        g_sb[:, : 2 * sz], o_ps[:, : 2 * sz], act, scale=1.0
                    )
                    nc.vector.tensor_add(
                        o_sb[:, j0 * sz : (j0 + 2) * sz],
                        g_sb[:, : 2 * sz],
                        x_sb[:, _xcol(h, j0) : _xcol(h, j0) + 2 * sz],
                    )
                if h == 0:
                    # mid-kernel: one big store, 3KB descriptor rows
                    nc.gpsimd.dma_start(
                        outt[:, 0 : FCH * sz], o_sb[:, : FCH * sz]
                    )
                else:
                    # tail: split so the last store (gating the drain) is
                    # small and its data+completion latency short
                    nc.sync.dma_start(
                        outt[:, FCH * sz : FCH * sz + 4 * sz], o_sb[:, : 4 * sz]
                    )
                    nc.gpsimd.dma_start(
                        outt[:, FCH * sz + 4 * sz : 2 * FCH * sz],
                        o_sb[:, 4 * sz : FCH * sz],
                    )
                return first_mm

            # h1's PE work is emitted after phase2(0) so the strict PE FIFO
            # doesn't block half 0's output pipeline on half 1's loads.
            # Fillers sit in the PE FIFO where the engine would otherwise
            # idle waiting on the casts.
            mm1_half(0)
            warm(N_FILLER)
            first_mm_h0 = phase2_half(0)
            mm1_half(1, after=first_mm_h0)
            warm(N_FILLER)
            phase2_half(1)

    nc.finalize()
    return nc


def _get_program():
    if "nc" not in _CACHE:
        _CACHE["nc"] = _build_program()
    return _CACHE["nc"]


def _host_prep(hidden_states, bias, cores):
    """Collapse TT cores to rank-5 factors; pack consts + x^T per core."""
    c0, c1, c2, c3, c4, c5 = [c.astype(np.float64) for c in cores]
    A = np.einsum("iv,vjw,wkx->ijkx", c0[0], c1, c2).reshape(HID, RANK)
    Bm = np.einsum("xpy,yqz,zr->xpqr", c3, c4, c5[:, :, 0]).reshape(RANK, HID)

    a_p = np.ascontiguousarray(
        A.reshape(FCH, 128, RANK).transpose(1, 0, 2).reshape(128, FCH * RANK)
    ).astype(ml_dtypes.bfloat16)                   # (128, 30)

    xts = []
    for c in range(NCORES):
        xc = hidden_states[c]  # (512, 768)
        xct = xc.T.astype(ml_dtypes.bfloat16)  # (768, 512)
        # per half: [p, c*sz + m~] = x^T[c*128+p, off+m~]
        blocks = [a_p]
        for h in (0, 1):
            sz, off = HSIZE[h], HOFF[h]
            blocks.append(
                xct[:, off : off + sz]
                .reshape(FCH, 128, sz)
                .transpose(1, 0, 2)
                .reshape(128, FCH * sz)
            )
        bm_blk = np.zeros((128, HID), dtype=ml_dtypes.bfloat16)
        bm_blk[:RANK] = Bm.astype(ml_dtypes.bfloat16)
        bm_blk[32] = bias.astype(ml_dtypes.bfloat16)
        blocks.append(bm_blk)
        xts.append(np.ascontiguousarray(np.concatenate(blocks, axis=1)))
    return xts


def _unpack_out(outt_list):
    """outt[p, h*1536 + j*256 + m~] = out[h*256+m~, j*128+p] -> (8,512,768)."""
    outs = []
    for outt in outt_list:
        o = (
            outt.reshape(128, 2, FCH, HSIZE[0])
            .transpose(1, 3, 2, 0)
            .reshape(ROWS, HID)
        )
        outs.append(o)
    return np.stack(outs, axis=0).astype(np.float32)


def run(inputs, trace=False, **spmd_kwargs):
    hidden_states = np.asarray(inputs["hidden_states"], dtype=np.float32)
    bias = np.asarray(inputs["bias"], dtype=np.float32)
    cores = [np.asarray(inputs[f"core{i}"], dtype=np.float32) for i in range(6)]

    xts = _host_prep(hidden_states, bias, cores)
    nc = _get_program()
    in_maps = [{"xt": xts[c]} for c in range(NCORES)]
    res = run_bass_kernel_spmd(
        nc, in_maps, core_ids=list(range(NCORES)), trace=trace, **spmd_kwargs
    )
    out = _unpack_out([res.results[c]["outt"] for c in range(NCORES)])
    if trace:
        return out, res
    return out


def kernel(**inputs):
    return run(inputs)


# revision 18
# speedup vs baseline: 1.0731x; 1.0474x over previous
# Trainium2 Bass kernel for nn_BertAdapter_SLT_49933289783411
#
# Reference computation:
#   y   = tt_linear(x) + bias          (TT-factorized 768->768 linear)
#   out = x + gelu_exact(y)
#
# Key math: the TT cores with ranks [1,5,5,5,5,5,1] factor the 768x768
# weight as W = A @ B with A:(768,5), B:(5,768).  We precompute A,B on
# host (tiny, exact) and run a rank-5 bottleneck matmul on device.
#
# Sharding: data-parallel over the batch dim (8 batch elements -> 8 cores).
# Each core handles x_c:(512,768), pre-transposed on host to x^T (feature-
# major) so the contraction dim lands on SBUF partitions.  Per core:
#   t3    = A^T @ x^T              (5,512)   PSUM accumulate over f-chunks
#   y^T_j = B_j^T @ t3_pad         (128,512) per 128-feature output chunk j
#   o^T_j = x^T_j + gelu_exact(y^T_j + bias_j)
# The host transposes the gathered o^T back.
#
# The whole data path runs in bf16 (x load, mm1, mm2, gelu output,
# residual add, output store); accumulation stays f32 in PSUM.  The
# rel-err budget is 2e-2 and the bf16 path measures ~2.3e-3 end to end,
# while halving both DMA directions and quartering mm1's PE passes
# vs an f32 x path.  The host upcasts the bf16 output to f32.
#
# mm2 runs with K=33 partitions (5 TT ranks + bias row 32): matmul time
# depends only on N, so shrinking K is free, and B then needs only 6
# DMA'd rows instead of a 128-row zero-padded block.
#
# Structure for latency hiding: the 512 batch rows are processed as two
# halves.  Half 0's entire output pipeline (mm2 -> gelu -> residual ->
# store) runs while half 1's x is still streaming from HBM, hiding the
# DMA completion-semaphore latency and overlapping store with load
# traffic.

import numpy as np
import ml_dtypes

import concourse.bass as bass
import concourse.bacc as bacc
import concourse.mybir as mybir
import concourse.tile as tile
from concourse.tile import add_dep_helper
from concourse.bass_utils import run_bass_kernel_spmd

HID = 768
ROWS = 512          # rows per core (one batch element)
HSIZE = (256, 256)
HOFF = (0, 256)
NCORES = 8
FCH = 6             # 768 / 128 feature chunks
RANK = 5
F32 = mybir.dt.float32
BF16 = mybir.dt.bfloat16

N_WARMUP = 30       # dummy PE matmuls: sustained PE power draw trips the
                    # HAM clock un-throttle (~2x matmul rate) after ~3.9us
                    # of gap-free bf16 random-data activity (v5/v6 A/B:
                    # 28 contiguous warmups abutting mm1 tripped it at
                    # +3.9us; 12 warmups with a 0.8us gap before mm1 never
                    # tripped it).  Emitted BEFORE the TileContext so the
                    # PE starts as soon as its preamble ends (~6us) instead
                    # of after the tile entry barrier (~7.6us); sized to
                    # keep the FIFO busy until the first x chunks' DMA
                    # semaphore arrives (~10.3us, jitters +-0.7us).
N_FILLER = 5        # filler matmuls bridging the PE-idle cast-wait gaps
                    # (mm1 -> mm2 handoff) so the power integrator keeps
                    # climbing until the boost trips and doesn't decay
K2 = 33             # mm2 contraction: rows 0..4 = TT rank, row 32 = bias

# packed layout of the input tensor, in bf16 columns:
#   [A (128,30)] [x half0: c0..c5 x 256] [x half1: ...] [B (rows 0:5 + 32)]
A_COLS = FCH * RANK                                # 30
XH_COLS = FCH * HSIZE[0]                           # 1536
BM_OFF = A_COLS + 2 * XH_COLS                      # 3102
XT_COLS = BM_OFF + HID                             # 3870
OUT_COLS = FCH * ROWS                              # 3072

_CACHE = {}


class _LeanTileContext(tile.TileContext):
    """TileContext with a minimal exit sequence.

    The stock exit emits drain + all-engine barrier + per-sem clears +
    barrier (~2-3us).  The runtime re-initializes semaphore state on every
    NEFF execution (verified empirically: repeated executions of the same
    loaded executable stay bit-correct without the clears), so only the
    drain — which makes the kernel end wait for the output DMAs — is kept.
    """

    def _drain_and_barrier(self, tick_clock, wait_clock):
        drain_inst = self.nc.sync.drain()
        # Wait only on the DMA proc clocks (SWDGE+HWDGE, procs 11..26).
        # The stock global-clock wait spans all 27 procs and lowers to a
        # ~50-instruction EVENT_SEMAPHORE chain (~2us of tail).  Output
        # correctness only needs the store DMAs: every compute result the
        # stores depend on is upstream of those DMA sem increments, and
        # each engine's own queue end is awaited by the runtime anyway.
        gc = tick_clock.global_clock
        vals = [gc[p] if p >= 11 else 0 for p in range(27)]
        wait_clock.add_sem_waits(
            drain_inst.ins, tile.ScopedClock({None: tile.VectorClock(vals)})
        )
        popped = self.nc._tile_sem_poison_stack.pop()
        assert popped is self._sem_poison


def _xcol(h, c):
    return A_COLS + h * XH_COLS + c * HSIZE[h]


def _build_program(act=None):
    if act is None:
        act = mybir.ActivationFunctionType.Gelu
    nc = bacc.Bacc(None, target_bir_lowering=False)
    xt = nc.dram_tensor("xt", [128, XT_COLS], BF16, kind="ExternalInput")
    outt = nc.dram_tensor("outt", [128, OUT_COLS], BF16, kind="ExternalOutput")

    # --- pre-tile PE warmup: emitted before the TileContext so it is not
    # held back by the tile entry barrier.  Deliberately race-y: the
    # matmuls read wsb while the RNG fill may still be writing it — the
    # values are irrelevant (wps is never read), the matmuls only need
    # bit-toggling operands to draw power so the HAM clock gate opens.
    # The SBUF/PSUM space frees back to the tile pools below; that reuse
    # is safe because the PE FIFO executes all warmups before any tile
    # matmul, and only the PE touches PSUM.
    with (
        nc.sbuf_tensor("warm_sb", [128, 128], BF16) as wsb,
        nc.psum_tensor("warm_ps", [128, 128], F32) as wps,
    ):
        nc.gpsimd.random(wsb[:])
        for _ in range(N_WARMUP):
            nc.tensor.matmul(wps[:], wsb[:], wsb[:], start=True, stop=True)

    with _LeanTileContext(nc) as tc:
        with (
            tc.tile_pool(name="const", bufs=1) as cpool,
            tc.tile_pool(name="xs", bufs=1) as xpool,
            tc.tile_pool(name="work", bufs=4) as wpool,
            tc.tile_pool(name="ps_t3", bufs=1, space="PSUM") as tpool,
            tc.tile_pool(name="ps_o", bufs=4, space="PSUM") as opool,
            tc.tile_pool(name="ps_w", bufs=1, space="PSUM") as wps_pool,
        ):
            # filler-matmul scratch (random data, same power rationale)
            fsb = cpool.tile([128, 128], BF16)
            nc.vector.random(fsb[:])
            fps = wps_pool.tile([128, 128], F32)

            def warm(n):
                for _ in range(n):
                    nc.tensor.matmul(fps[:], fsb[:], fsb[:], start=True, stop=True)

            # t3 in bf16; rows 5..31 stay zero, row 32 is all-ones: paired
            # with the bias in B's row 32 it folds the TT bias into mm2
            # (ACT then needs no bias, so gelu can run on j-pairs in one op)
            t3_sb = cpool.tile([128, ROWS], BF16)
            # partition writes must be 32-aligned
            nc.gpsimd.memset(t3_sb[0:32, :], 0.0)
            nc.gpsimd.memset(t3_sb[32:64, :], 1.0)

            x_sb = xpool.tile([128, XT_COLS], BF16)
            a_view = x_sb[:, 0:A_COLS]                    # (128,30)
            bm_view = x_sb[:, BM_OFF:XT_COLS]             # (128,768); rows 0:33 used

            # B rows 5..32 must read as zero for the K=33 mm2
            nc.vector.memset(x_sb[0:32, BM_OFF:XT_COLS], 0.0)

            t3_ps = [
                tpool.tile([RANK, HSIZE[h]], F32, name=f"t3_ps{h}") for h in (0, 1)
            ]

            # x loads: 4 serial DMAs on the Scalar queue (3 chunks each, the
            # first also carries A).  Serial beats parallel queues here: the
            # first chunks' data lands earliest when it has the full HBM
            # bandwidth, and mm1 is chunk-gated.  Scalar (not Sync) because
            # its sequencer reaches 'main' ~0.9us earlier — Sync's preamble
            # has an extra long drain — and the ACT table loads run on the
            # ACT unit concurrently with the DMA issues.
            for d in range(4):
                start = 0 if d == 0 else A_COLS + d * 3 * HSIZE[0]
                end = A_COLS + (d + 1) * 3 * HSIZE[0]
                nc.scalar.dma_start(x_sb[:, start:end], xt[:, start:end])
            # B rows + bias row on the Sync queue (tiny, lands early)
            nc.sync.dma_start(
                x_sb[0:RANK, BM_OFF:XT_COLS], xt[0:RANK, BM_OFF:XT_COLS]
            )
            nc.sync.dma_start(
                x_sb[32:33, BM_OFF:XT_COLS], xt[32:33, BM_OFF:XT_COLS]
            )

            def mm1_half(h, after=None):
                for c in range(FCH):
                    mm = nc.tensor.matmul(
                        t3_ps[h][:],
                        a_view[:, c * RANK : (c + 1) * RANK],
                        x_sb[:, _xcol(h, c) : _xcol(h, c) + HSIZE[h]],
                        start=(c == 0),
                        stop=(c == FCH - 1),
                    )
                    if after is not None:
                        # ordering-only edge: keep these DMA-gated matmuls
                        # out of the strict PE FIFO until half 0's mm2s ran
                        add_dep_helper(
                            mm.ins, after.ins, sync=False,
                            reason="mm1 h1 after phase2 h0 matmuls",
                        )

            def phase2_half(h):
                sz, off = HSIZE[h], HOFF[h]
                nc.vector.tensor_copy(t3_sb[0:RANK, off : off + sz], t3_ps[h][:])
                first_mm = None
                # the whole half's output accumulates into one tile so a
                # single store moves it with 3KB descriptor rows (1KB rows
                # measured ~2.5x slower per byte)
                o_sb = wpool.tile([128, FCH * max(HSIZE)], BF16, tag="o_sb", bufs=2)
                for j0 in range(0, FCH, 2):
                    # two output chunks share one PSUM bank: the first matmul
                    # (start=True) clears the bank's has_written bits, the
                    # second (start=False) overwrites its still-clear region
                    o_ps = opool.tile([128, 2 * max(HSIZE)], F32, tag="o_ps")
                    for k in (0, 1):
                        mm = nc.tensor.matmul(
                            o_ps[:, k * sz : (k + 1) * sz],
                            bm_view[0:K2, (j0 + k) * 128 : (j0 + k + 1) * 128],
                            t3_sb[0:K2, off : off + sz],
                            start=(k == 0),
                            stop=(k == 1),
                        )
                        if first_mm is None:
                            first_mm = mm
                    # one paired gelu halves the per-op ACT overhead on the
                    # critical tail (bias already folded in via mm2)
                    g_sb = wpool.tile([128, 2 * max(HSIZE)], BF16, tag="g_sb", bufs=4)
                    nc.scalar.activation(
                        g_sb[:, : 2 * sz], o_ps[:, : 2 * sz], act, scale=1.0
                    )
                    nc.vector.tensor_add(
                        o_sb[:, j0 * sz : (j0 + 2) * sz],
                        g_sb[:, : 2 * sz],
                        x_sb[:, _xcol(h, j0) : _xcol(h, j0) + 2 * sz],
                    )
                if h == 0:
                    # mid-kernel: one big store, 3KB descriptor rows
                    nc.gpsimd.dma_start(
                        outt[:, 0 : FCH * sz], o_sb[:, : FCH * sz]
                    )
                else:
                    # tail: split so the last store (gating the drain) is
                    # small and its data+completion latency short
                    nc.sync.dma_start(
                        outt[:, FCH * sz : FCH * sz + 4 * sz], o_sb[:, : 4 * sz]
                    )
                    nc.gpsimd.dma_start(
                        outt[:, FCH * sz + 4 * sz : 2 * FCH * sz],
                        o_sb[:, 4 * sz : FCH * sz],
                    )
                return first_mm

            # h1's PE work is emitted after phase2(0) so the strict PE FIFO
            # doesn't block half 0's output pipeline on half 1's loads.
            # Fillers sit in the PE FIFO where the engine would otherwise
            # idle waiting on the casts.
            mm1_half(0)
            warm(N_FILLER)
            first_mm_h0 = phase2_half(0)
            mm1_half(1, after=first_mm_h0)
            warm(N_FILLER)
            phase2_half(1)

    nc.finalize()
    return nc


def _get_program():
    if "nc" not in _CACHE:
        _CACHE["nc"] = _build_program()
    return _CACHE["nc"]


def _host_prep(hidden_states, bias, cores):
    """Collapse TT cores to rank-5 factors; pack consts + x^T per core."""
    c0, c1, c2, c3, c4, c5 = [c.astype(np.float64) for c in cores]
    A = np.einsum("iv,vjw,wkx->ijkx", c0[0], c1, c2).reshape(HID, RANK)
    Bm = np.einsum("xpy,yqz,zr->xpqr", c3, c4, c5[:, :, 0]).reshape(RANK, HID)

    a_p = np.ascontiguousarray(
        A.reshape(FCH, 128, RANK).transpose(1, 0, 2).reshape(128, FCH * RANK)
    ).astype(ml_dtypes.bfloat16)                   # (128, 30)

    xts = []
    for c in range(NCORES):
        xc = hidden_states[c]  # (512, 768)
        xct = xc.T.astype(ml_dtypes.bfloat16)  # (768, 512)
        # per half: [p, c*sz + m~] = x^T[c*128+p, off+m~]
        blocks = [a_p]
        for h in (0, 1):
            sz, off = HSIZE[h], HOFF[h]
            blocks.append(
                xct[:, off : off + sz]
                .reshape(FCH, 128, sz)
                .transpose(1, 0, 2)
                .reshape(128, FCH * sz)
            )
        bm_blk = np.zeros((128, HID), dtype=ml_dtypes.bfloat16)
        bm_blk[:RANK] = Bm.astype(ml_dtypes.bfloat16)
        bm_blk[32] = bias.astype(ml_dtypes.bfloat16)
        blocks.append(bm_blk)
        xts.append(np.ascontiguousarray(np.concatenate(blocks, axis=1)))
    return xts


def _unpack_out(outt_list):
    """outt[p, h*1536 + j*256 + m~] = out[h*256+m~, j*128+p] -> (8,512,768)."""
    outs = []
    for outt in outt_list:
        o = (
            outt.reshape(128, 2, FCH, HSIZE[0])
            .transpose(1, 3, 2, 0)
            .reshape(ROWS, HID)
        )
        outs.append(o)
    return np.stack(outs, axis=0).astype(np.float32)


def run(inputs, trace=False, **spmd_kwargs):
    hidden_states = np.asarray(inputs["hidden_states"], dtype=np.float32)
    bias = np.asarray(inputs["bias"], dtype=np.float32)
    cores = [np.asarray(inputs[f"core{i}"], dtype=np.float32) for i in range(6)]

    xts = _host_prep(hidden_states, bias, cores)
    nc = _get_program()
    in_maps = [{"xt": xts[c]} for c in range(NCORES)]
    res = run_bass_kernel_spmd(
        nc, in_maps, core_ids=list(range(NCORES)), trace=trace, **spmd_kwargs
    )
    out = _unpack_out([res.results[c]["outt"] for c in range(NCORES)])
    if trace:
        return out, res
    return out


def kernel(**inputs):
    return run(inputs)


# revision 23
# speedup vs baseline: 1.0915x; 1.0171x over previous
# Trainium2 Bass kernel for nn_BertAdapter_SLT_49933289783411
#
# Reference computation:
#   y   = tt_linear(x) + bias          (TT-factorized 768->768 linear)
#   out = x + gelu_exact(y)
#
# Key math: the TT cores with ranks [1,5,5,5,5,5,1] factor the 768x768
# weight as W = A @ B with A:(768,5), B:(5,768).  We precompute A,B on
# host (tiny, exact) and run a rank-5 bottleneck matmul on device.
#
# Sharding: data-parallel over the batch dim (8 batch elements -> 8 cores).
# Each core handles x_c:(512,768), pre-transposed on host to x^T (feature-
# major) so the contraction dim lands on SBUF partitions.  Per core:
#   t3    = A^T @ x^T              (5,512)   PSUM accumulate over f-chunks
#   y^T_j = B_j^T @ t3_pad         (128,512) per 128-feature output chunk j
#   o^T_j = x^T_j + gelu_exact(y^T_j + bias_j)
# The host transposes the gathered o^T back.
#
# The whole data path runs in bf16 (x load, mm1, mm2, gelu output,
# residual add, output store); accumulation stays f32 in PSUM.  The
# rel-err budget is 2e-2 and the bf16 path measures ~2.3e-3 end to end,
# while halving both DMA directions and quartering mm1's PE passes
# vs an f32 x path.  The host upcasts the bf16 output to f32.
#
# mm2 runs with K=33 partitions (5 TT ranks + bias row 32): matmul time
# depends only on N, so shrinking K is free, and B then needs only 6
# DMA'd rows instead of a 128-row zero-padded block.
#
# Structure for latency hiding: the 512 batch rows are processed as two
# halves.  Half 0's entire output pipeline (mm2 -> gelu -> residual ->
# store) runs while half 1's x is still streaming from HBM, hiding the
# DMA completion-semaphore latency and overlapping store with load
# traffic.

import numpy as np
import ml_dtypes

import concourse.bass as bass
import concourse.bacc as bacc
import concourse.mybir as mybir
import concourse.tile as tile
from concourse.tile import add_dep_helper
from concourse.bass_utils import run_bass_kernel_spmd

HID = 768
ROWS = 512          # rows per core (one batch element)
HSIZE = (256, 256)
HOFF = (0, 256)
NCORES = 8
FCH = 6             # 768 / 128 feature chunks
RANK = 5
F32 = mybir.dt.float32
BF16 = mybir.dt.bfloat16

N_WARMUP = 30       # dummy PE matmuls: sustained PE power draw trips the
                    # HAM clock un-throttle (~2x matmul rate) after ~3.9us
                    # of gap-free bf16 random-data activity (v5/v6 A/B:
                    # 28 contiguous warmups abutting mm1 tripped it at
                    # +3.9us; 12 warmups with a 0.8us gap before mm1 never
                    # tripped it).  Emitted BEFORE the TileContext so the
                    # PE starts as soon as its preamble ends (~6us) instead
                    # of after the tile entry barrier (~7.6us); sized to
                    # keep the FIFO busy until the first x chunks' DMA
                    # semaphore arrives (~10.3us, jitters +-0.7us).
K2 = 33             # mm2 contraction: rows 0..4 = TT rank, row 32 = bias

# packed layout of the input tensor, in bf16 columns:
#   [A (128,30)] [x half0: c0..c5 x 256] [x half1: ...] [B (rows 0:5 + 32)]
A_COLS = FCH * RANK                                # 30
XH_COLS = FCH * HSIZE[0]                           # 1536
BM_OFF = A_COLS + 2 * XH_COLS                      # 3102
XT_COLS = BM_OFF + HID                             # 3870
OUT_COLS = FCH * ROWS                              # 3072

_CACHE = {}


class _LeanTileContext(tile.TileContext):
    """TileContext with a minimal exit sequence.

    The stock exit emits drain + all-engine barrier + per-sem clears +
    barrier (~2-3us).  The runtime re-initializes semaphore state on every
    NEFF execution (verified empirically: repeated executions of the same
    loaded executable stay bit-correct without the clears), so only the
    drain — which makes the kernel end wait for the output DMAs — is kept.
    """

    def _drain_and_barrier(self, tick_clock, wait_clock):
        drain_inst = self.nc.sync.drain()
        # Wait only on the DMA proc clocks (SWDGE+HWDGE, procs 11..26).
        # The stock global-clock wait spans all 27 procs and lowers to a
        # ~50-instruction EVENT_SEMAPHORE chain (~2us of tail).  Output
        # correctness only needs the store DMAs: every compute result the
        # stores depend on is upstream of those DMA sem increments, and
        # each engine's own queue end is awaited by the runtime anyway.
        gc = tick_clock.global_clock
        vals = [gc[p] if p >= 11 else 0 for p in range(27)]
        wait_clock.add_sem_waits(
            drain_inst.ins, tile.ScopedClock({None: tile.VectorClock(vals)})
        )
        popped = self.nc._tile_sem_poison_stack.pop()
        assert popped is self._sem_poison


def _xcol(h, c):
    return A_COLS + h * XH_COLS + c * HSIZE[h]


def _build_program(act=None):
    if act is None:
        act = mybir.ActivationFunctionType.Gelu
    nc = bacc.Bacc(None, target_bir_lowering=False)
    xt = nc.dram_tensor("xt", [128, XT_COLS], BF16, kind="ExternalInput")
    outt = nc.dram_tensor("outt", [128, OUT_COLS], BF16, kind="ExternalOutput")

    # --- pre-tile PE warmup: emitted before the TileContext so it is not
    # held back by the tile entry barrier.  Deliberately race-y: the
    # matmuls read wsb while the RNG fill may still be writing it — the
    # values are irrelevant (wps is never read), the matmuls only need
    # bit-toggling operands to draw power so the HAM clock gate opens.
    # The SBUF/PSUM space frees back to the tile pools below; that reuse
    # is safe because the PE FIFO executes all warmups before any tile
    # matmul, and only the PE touches PSUM.
    with (
        nc.sbuf_tensor("warm_sb", [128, 128], BF16) as wsb,
        nc.psum_tensor("warm_ps", [128, 128], F32) as wps,
    ):
        nc.gpsimd.random(wsb[:])
        for _ in range(N_WARMUP):
            nc.tensor.matmul(wps[:], wsb[:], wsb[:], start=True, stop=True)

    with _LeanTileContext(nc) as tc:
        with (
            tc.tile_pool(name="const", bufs=1) as cpool,
            tc.tile_pool(name="xs", bufs=1) as xpool,
            tc.tile_pool(name="work", bufs=4) as wpool,
            tc.tile_pool(name="ps_t3", bufs=1, space="PSUM") as tpool,
            tc.tile_pool(name="ps_o", bufs=4, space="PSUM") as opool,
        ):
            # t3 in bf16; rows 5..31 stay zero, row 32 is all-ones: paired
            # with the bias in B's row 32 it folds the TT bias into mm2
            # (ACT then needs no bias, so gelu can run on j-pairs in one op)
            t3_sb = cpool.tile([128, ROWS], BF16)
            # partition writes must be 32-aligned
            nc.gpsimd.memset(t3_sb[0:32, :], 0.0)
            nc.gpsimd.memset(t3_sb[32:64, :], 1.0)

            x_sb = xpool.tile([128, XT_COLS], BF16)
            a_view = x_sb[:, 0:A_COLS]                    # (128,30)
            bm_view = x_sb[:, BM_OFF:XT_COLS]             # (128,768); rows 0:33 used

            # B rows 5..32 must read as zero for the K=33 mm2
            nc.vector.memset(x_sb[0:32, BM_OFF:XT_COLS], 0.0)

            t3_ps = [
                tpool.tile([RANK, HSIZE[h]], F32, name=f"t3_ps{h}") for h in (0, 1)
            ]

            # x loads: 4 serial DMAs on the Scalar queue (3 chunks each, the
            # first also carries A).  Serial beats parallel queues here: the
            # first chunks' data lands earliest when it has the full HBM
            # bandwidth, and mm1 is chunk-gated.  Scalar (not Sync) because
            # its sequencer reaches 'main' ~0.9us earlier — Sync's preamble
            # has an extra long drain — and the ACT table loads run on the
            # ACT unit concurrently with the DMA issues.
            for d in range(4):
                start = 0 if d == 0 else A_COLS + d * 3 * HSIZE[0]
                end = A_COLS + (d + 1) * 3 * HSIZE[0]
                nc.scalar.dma_start(x_sb[:, start:end], xt[:, start:end])
            # B rows + bias row on the Sync queue (tiny, lands early)
            nc.sync.dma_start(
                x_sb[0:RANK, BM_OFF:XT_COLS], xt[0:RANK, BM_OFF:XT_COLS]
            )
            nc.sync.dma_start(
                x_sb[32:33, BM_OFF:XT_COLS], xt[32:33, BM_OFF:XT_COLS]
            )

            def mm1_half(h):
                last = None
                for c in range(FCH):
                    last = nc.tensor.matmul(
                        t3_ps[h][:],
                        a_view[:, c * RANK : (c + 1) * RANK],
                        x_sb[:, _xcol(h, c) : _xcol(h, c) + HSIZE[h]],
                        start=(c == 0),
                        stop=(c == FCH - 1),
                    )
                return last

            def cast_half(h):
                sz, off = HSIZE[h], HOFF[h]
                nc.vector.tensor_copy(t3_sb[0:RANK, off : off + sz], t3_ps[h][:])

            def phase2_half(h, after=None):
                sz, off = HSIZE[h], HOFF[h]
                first_mm = None
                # the whole half's output accumulates into one tile so a
                # single store moves it with 3KB descriptor rows (1KB rows
                # measured ~2.5x slower per byte)
                o_sb = wpool.tile([128, FCH * max(HSIZE)], BF16, tag="o_sb", bufs=2)
                for j0 in range(0, FCH, 2):
                    # two output chunks share one PSUM bank: the first matmul
                    # (start=True) clears the bank's has_written bits, the
                    # second (start=False) overwrites its still-clear region
                    o_ps = opool.tile([128, 2 * max(HSIZE)], F32, tag="o_ps")
                    for k in (0, 1):
                        mm = nc.tensor.matmul(
                            o_ps[:, k * sz : (k + 1) * sz],
                            bm_view[0:K2, (j0 + k) * 128 : (j0 + k + 1) * 128],
                            t3_sb[0:K2, off : off + sz],
                            start=(k == 0),
                            stop=(k == 1),
                        )
                        if first_mm is None:
                            first_mm = mm
                        if after is not None:
                            # ordering-only edge: pin the PE FIFO order so
                            # both halves' mm1 run back-to-back before any
                            # mm2 — the casts then overlap PE work and the
                            # ACT engine gets a gap-free gelu run
                            add_dep_helper(
                                mm.ins, after.ins, sync=False,
                                reason="mm2 after both mm1 halves",
                            )
                            after = None
                    # one paired gelu halves the per-op ACT overhead on the
                    # critical tail (bias already folded in via mm2)
                    g_sb = wpool.tile([128, 2 * max(HSIZE)], BF16, tag="g_sb", bufs=4)
                    nc.scalar.activation(
                        g_sb[:, : 2 * sz], o_ps[:, : 2 * sz], act, scale=1.0
                    )
                    nc.vector.tensor_add(
                        o_sb[:, j0 * sz : (j0 + 2) * sz],
                        g_sb[:, : 2 * sz],
                        x_sb[:, _xcol(h, j0) : _xcol(h, j0) + 2 * sz],
                    )
                if h == 0:
                    # mid-kernel: one big store, 3KB descriptor rows
                    nc.gpsimd.dma_start(
                        outt[:, 0 : FCH * sz], o_sb[:, : FCH * sz]
                    )
                else:
                    # tail: split so the last store (gating the drain) is
                    # small and its data+completion latency short
                    nc.sync.dma_start(
                        outt[:, FCH * sz : FCH * sz + 4 * sz], o_sb[:, : 4 * sz]
                    )
                    nc.gpsimd.dma_start(
                        outt[:, FCH * sz + 4 * sz : 2 * FCH * sz],
                        o_sb[:, 4 * sz : FCH * sz],
                    )
                return first_mm

            # PE FIFO order: mm1 h0, mm1 h1, mm2 h0, mm2 h1.  Both casts
            # run on the DVE while the PE is still doing mm1/mm2 work, so
            # the PSUM->SBUF round-trip latency leaves the critical path,
            # and the ACT engine's six gelus run with no producer gap.
            mm1_half(0)
            last_mm1_h1 = mm1_half(1)
            cast_half(0)
            cast_half(1)
            phase2_half(0, after=last_mm1_h1)
            phase2_half(1)

    nc.finalize()
    return nc


def _get_program():
    if "nc" not in _CACHE:
        _CACHE["nc"] = _build_program()
    return _CACHE["nc"]


def _host_prep(hidden_states, bias, cores):
    """Collapse TT cores to rank-5 factors; pack consts + x^T per core."""
    c0, c1, c2, c3, c4, c5 = [c.astype(np.float64) for c in cores]
    A = np.einsum("iv,vjw,wkx->ijkx", c0[0], c1, c2).reshape(HID, RANK)
    Bm = np.einsum("xpy,yqz,zr->xpqr", c3, c4, c5[:, :, 0]).reshape(RANK, HID)

    a_p = np.ascontiguousarray(
        A.reshape(FCH, 128, RANK).transpose(1, 0, 2).reshape(128, FCH * RANK)
    ).astype(ml_dtypes.bfloat16)                   # (128, 30)

    xts = []
    for c in range(NCORES):
        xc = hidden_states[c]  # (512, 768)
        xct = xc.T.astype(ml_dtypes.bfloat16)  # (768, 512)
        # per half: [p, c*sz + m~] = x^T[c*128+p, off+m~]
        blocks = [a_p]
        for h in (0, 1):
            sz, off = HSIZE[h], HOFF[h]
            blocks.append(
                xct[:, off : off + sz]
                .reshape(FCH, 128, sz)
                .transpose(1, 0, 2)
                .reshape(128, FCH * sz)
            )
        bm_blk = np.zeros((128, HID), dtype=ml_dtypes.bfloat16)
        bm_blk[:RANK] = Bm.astype(ml_dtypes.bfloat16)
        bm_blk[32] = bias.astype(ml_dtypes.bfloat16)
        blocks.append(bm_blk)
        xts.append(np.ascontiguousarray(np.concatenate(blocks, axis=1)))
    return xts


def _unpack_out(outt_list):
    """outt[p, h*1536 + j*256 + m~] = out[h*256+m~, j*128+p] -> (8,512,768)."""
    outs = []
    for outt in outt_list:
        o = (
            outt.reshape(128, 2, FCH, HSIZE[0])
            .transpose(1, 3, 2, 0)
            .reshape(ROWS, HID)
        )
        outs.append(o)
    return np.stack(outs, axis=0).astype(np.float32)


def run(inputs, trace=False, **spmd_kwargs):
    hidden_states = np.asarray(inputs["hidden_states"], dtype=np.float32)
    bias = np.asarray(inputs["bias"], dtype=np.float32)
    cores = [np.asarray(inputs[f"core{i}"], dtype=np.float32) for i in range(6)]

    xts = _host_prep(hidden_states, bias, cores)
    nc = _get_program()
    in_maps = [{"xt": xts[c]} for c in range(NCORES)]
    res = run_bass_kernel_spmd(
        nc, in_maps, core_ids=list(range(NCORES)), trace=trace, **spmd_kwargs
    )
    out = _unpack_out([res.results[c]["outt"] for c in range(NCORES)])
    if trace:
        return out, res
    return out


def kernel(**inputs):
    return run(inputs)


# revision 24
# speedup vs baseline: 1.0935x; 1.0019x over previous
# Trainium2 Bass kernel for nn_BertAdapter_SLT_49933289783411
#
# Reference computation:
#   y   = tt_linear(x) + bias          (TT-factorized 768->768 linear)
#   out = x + gelu_exact(y)
#
# Key math: the TT cores with ranks [1,5,5,5,5,5,1] factor the 768x768
# weight as W = A @ B with A:(768,5), B:(5,768).  We precompute A,B on
# host (tiny, exact) and run a rank-5 bottleneck matmul on device.
#
# Sharding: data-parallel over the batch dim (8 batch elements -> 8 cores).
# Each core handles x_c:(512,768), pre-transposed on host to x^T (feature-
# major) so the contraction dim lands on SBUF partitions.  Per core:
#   t3    = A^T @ x^T              (5,512)   PSUM accumulate over f-chunks
#   y^T_j = B_j^T @ t3_pad         (128,512) per 128-feature output chunk j
#   o^T_j = x^T_j + gelu_exact(y^T_j + bias_j)
# The host transposes the gathered o^T back.
#
# The whole data path runs in bf16 (x load, mm1, mm2, gelu output,
# residual add, output store); accumulation stays f32 in PSUM.  The
# rel-err budget is 2e-2 and the bf16 path measures ~2.3e-3 end to end,
# while halving both DMA directions and quartering mm1's PE passes
# vs an f32 x path.  The host upcasts the bf16 output to f32.
#
# mm2 runs with K=33 partitions (5 TT ranks + bias row 32): matmul time
# depends only on N, so shrinking K is free, and B then needs only 6
# DMA'd rows instead of a 128-row zero-padded block.
#
# Structure for latency hiding: the 512 batch rows are processed as two
# halves.  Half 0's entire output pipeline (mm2 -> gelu -> residual ->
# store) runs while half 1's x is still streaming from HBM, hiding the
# DMA completion-semaphore latency and overlapping store with load
# traffic.

import numpy as np
import ml_dtypes

import concourse.bass as bass
import concourse.bacc as bacc
import concourse.mybir as mybir
import concourse.tile as tile
from concourse.tile import add_dep_helper
from concourse.bass_utils import run_bass_kernel_spmd

HID = 768
ROWS = 512          # rows per core (one batch element)
HSIZE = (256, 256)
HOFF = (0, 256)
NCORES = 8
FCH = 6             # 768 / 128 feature chunks
RANK = 5
F32 = mybir.dt.float32
BF16 = mybir.dt.bfloat16

N_WARMUP = 34       # dummy PE matmuls: sustained PE power draw trips the
                    # HAM clock un-throttle (~2x matmul rate) after ~3.9us
                    # of gap-free bf16 random-data activity (v5/v6 A/B:
                    # 28 contiguous warmups abutting mm1 tripped it at
                    # +3.9us; 12 warmups with a 0.8us gap before mm1 never
                    # tripped it).  Emitted BEFORE the TileContext so the
                    # PE starts as soon as its preamble ends (~6us) instead
                    # of after the tile entry barrier (~7.6us); sized to
                    # keep the FIFO busy until the first x chunks' DMA
                    # semaphore arrives (~10.3us, jitters +-0.7us).
K2 = 33             # mm2 contraction: rows 0..4 = TT rank, row 32 = bias

# packed layout of the input tensor, in bf16 columns:
#   [A (128,30)] [x half0: c0..c5 x 256] [x half1: ...] [B (rows 0:5 + 32)]
A_COLS = FCH * RANK                                # 30
XH_COLS = FCH * HSIZE[0]                           # 1536
BM_OFF = A_COLS + 2 * XH_COLS                      # 3102
XT_COLS = BM_OFF + HID                             # 3870
OUT_COLS = FCH * ROWS                              # 3072

_CACHE = {}


class _LeanTileContext(tile.TileContext):
    """TileContext with a minimal exit sequence.

    The stock exit emits drain + all-engine barrier + per-sem clears +
    barrier (~2-3us).  The runtime re-initializes semaphore state on every
    NEFF execution (verified empirically: repeated executions of the same
    loaded executable stay bit-correct without the clears), so only the
    drain — which makes the kernel end wait for the output DMAs — is kept.
    """

    def _drain_and_barrier(self, tick_clock, wait_clock):
        drain_inst = self.nc.sync.drain()
        # Wait only on the DMA proc clocks (SWDGE+HWDGE, procs 11..26).
        # The stock global-clock wait spans all 27 procs and lowers to a
        # ~50-instruction EVENT_SEMAPHORE chain (~2us of tail).  Output
        # correctness only needs the store DMAs: every compute result the
        # stores depend on is upstream of those DMA sem increments, and
        # each engine's own queue end is awaited by the runtime anyway.
        gc = tick_clock.global_clock
        vals = [gc[p] if p >= 11 else 0 for p in range(27)]
        wait_clock.add_sem_waits(
            drain_inst.ins, tile.ScopedClock({None: tile.VectorClock(vals)})
        )
        popped = self.nc._tile_sem_poison_stack.pop()
        assert popped is self._sem_poison


def _xcol(h, c):
    return A_COLS + h * XH_COLS + c * HSIZE[h]


def _build_program(act=None):
    if act is None:
        act = mybir.ActivationFunctionType.Gelu
    nc = bacc.Bacc(None, target_bir_lowering=False)
    xt = nc.dram_tensor("xt", [128, XT_COLS], BF16, kind="ExternalInput")
    outt = nc.dram_tensor("outt", [128, OUT_COLS], BF16, kind="ExternalOutput")

    # --- pre-tile PE warmup: emitted before the TileContext so it is not
    # held back by the tile entry barrier.  Deliberately race-y: the
    # matmuls read wsb while the RNG fill may still be writing it — the
    # values are irrelevant (wps is never read), the matmuls only need
    # bit-toggling operands to draw power so the HAM clock gate opens.
    # The SBUF/PSUM space frees back to the tile pools below; that reuse
    # is safe because the PE FIFO executes all warmups before any tile
    # matmul, and only the PE touches PSUM.
    with (
        nc.sbuf_tensor("warm_sb", [128, 128], BF16) as wsb,
        nc.psum_tensor("warm_ps", [128, 128], F32) as wps,
    ):
        nc.gpsimd.random(wsb[:])
        for _ in range(N_WARMUP):
            nc.tensor.matmul(wps[:], wsb[:], wsb[:], start=True, stop=True)

    with _LeanTileContext(nc) as tc:
        with (
            tc.tile_pool(name="const", bufs=1) as cpool,
            tc.tile_pool(name="xs", bufs=1) as xpool,
            tc.tile_pool(name="work", bufs=4) as wpool,
            tc.tile_pool(name="ps_t3", bufs=1, space="PSUM") as tpool,
            tc.tile_pool(name="ps_o", bufs=4, space="PSUM") as opool,
        ):
            # t3 in bf16; rows 5..31 stay zero, row 32 is all-ones: paired
            # with the bias in B's row 32 it folds the TT bias into mm2
            # (ACT then needs no bias, so gelu can run on j-pairs in one op)
            t3_sb = cpool.tile([128, ROWS], BF16)
            # partition writes must be 32-aligned
            nc.gpsimd.memset(t3_sb[0:32, :], 0.0)
            nc.gpsimd.memset(t3_sb[32:64, :], 1.0)

            x_sb = xpool.tile([128, XT_COLS], BF16)
            a_view = x_sb[:, 0:A_COLS]                    # (128,30)
            bm_view = x_sb[:, BM_OFF:XT_COLS]             # (128,768); rows 0:33 used

            # B rows 5..32 must read as zero for the K=33 mm2
            nc.vector.memset(x_sb[0:32, BM_OFF:XT_COLS], 0.0)

            t3_ps = [
                tpool.tile([RANK, HSIZE[h]], F32, name=f"t3_ps{h}") for h in (0, 1)
            ]

            # x loads: 4 serial DMAs on the Scalar queue (3 chunks each, the
            # first also carries A).  Serial beats parallel queues here: the
            # first chunks' data lands earliest when it has the full HBM
            # bandwidth, and mm1 is chunk-gated.  Scalar (not Sync) because
            # its sequencer reaches 'main' ~0.9us earlier — Sync's preamble
            # has an extra long drain — and the ACT table loads run on the
            # ACT unit concurrently with the DMA issues.
            for d in range(4):
                start = 0 if d == 0 else A_COLS + d * 3 * HSIZE[0]
                end = A_COLS + (d + 1) * 3 * HSIZE[0]
                nc.scalar.dma_start(x_sb[:, start:end], xt[:, start:end])
            # B rows + bias row on the Sync queue (tiny, lands early)
            nc.sync.dma_start(
                x_sb[0:RANK, BM_OFF:XT_COLS], xt[0:RANK, BM_OFF:XT_COLS]
            )
            nc.sync.dma_start(
                x_sb[32:33, BM_OFF:XT_COLS], xt[32:33, BM_OFF:XT_COLS]
            )

            def mm1_half(h):
                last = None
                for c in range(FCH):
                    last = nc.tensor.matmul(
                        t3_ps[h][:],
                        a_view[:, c * RANK : (c + 1) * RANK],
                        x_sb[:, _xcol(h, c) : _xcol(h, c) + HSIZE[h]],
                        start=(c == 0),
                        stop=(c == FCH - 1),
                    )
                return last

            def cast_half(h):
                sz, off = HSIZE[h], HOFF[h]
                nc.vector.tensor_copy(t3_sb[0:RANK, off : off + sz], t3_ps[h][:])

            def phase2_half(h, after=None):
                sz, off = HSIZE[h], HOFF[h]
                first_mm = None
                # the whole half's output accumulates into one tile so a
                # single store moves it with 3KB descriptor rows (1KB rows
                # measured ~2.5x slower per byte)
                o_sb = wpool.tile([128, FCH * max(HSIZE)], BF16, tag="o_sb", bufs=2)
                for j0 in range(0, FCH, 2):
                    # two output chunks share one PSUM bank: the first matmul
                    # (start=True) clears the bank's has_written bits, the
                    # second (start=False) overwrites its still-clear region
                    o_ps = opool.tile([128, 2 * max(HSIZE)], F32, tag="o_ps")
                    for k in (0, 1):
                        mm = nc.tensor.matmul(
                            o_ps[:, k * sz : (k + 1) * sz],
                            bm_view[0:K2, (j0 + k) * 128 : (j0 + k + 1) * 128],
                            t3_sb[0:K2, off : off + sz],
                            start=(k == 0),
                            stop=(k == 1),
                        )
                        if first_mm is None:
                            first_mm = mm
                        if after is not None:
                            # ordering-only edge: pin the PE FIFO order so
                            # both halves' mm1 run back-to-back before any
                            # mm2 — the casts then overlap PE work and the
                            # ACT engine gets a gap-free gelu run
                            add_dep_helper(
                                mm.ins, after.ins, sync=False,
                                reason="mm2 after both mm1 halves",
                            )
                            after = None
                    # one paired gelu halves the per-op ACT overhead on the
                    # critical tail (bias already folded in via mm2)
                    g_sb = wpool.tile([128, 2 * max(HSIZE)], BF16, tag="g_sb", bufs=4)
                    nc.scalar.activation(
                        g_sb[:, : 2 * sz], o_ps[:, : 2 * sz], act, scale=1.0
                    )
                    nc.vector.tensor_add(
                        o_sb[:, j0 * sz : (j0 + 2) * sz],
                        g_sb[:, : 2 * sz],
                        x_sb[:, _xcol(h, j0) : _xcol(h, j0) + 2 * sz],
                    )
                if h == 0:
                    # mid-kernel: one big store, 3KB descriptor rows
                    nc.gpsimd.dma_start(
                        outt[:, 0 : FCH * sz], o_sb[:, : FCH * sz]
                    )
                else:
                    # tail: split so the last store (gating the drain) is
                    # small and its data+completion latency short
                    nc.sync.dma_start(
                        outt[:, FCH * sz : FCH * sz + 4 * sz], o_sb[:, : 4 * sz]
                    )
                    nc.gpsimd.dma_start(
                        outt[:, FCH * sz + 4 * sz : 2 * FCH * sz],
                        o_sb[:, 4 * sz : FCH * sz],
                    )
                return first_mm

            # PE FIFO order: mm1 h0, mm1 h1, mm2 h0, mm2 h1.  Both casts
            # run on the DVE while the PE is still doing mm1/mm2 work, so
            # the PSUM->SBUF round-trip latency leaves the critical path,
            # and the ACT engine's six gelus run with no producer gap.
            mm1_half(0)
            last_mm1_h1 = mm1_half(1)
            cast_half(0)
            cast_half(1)
            phase2_half(0, after=last_mm1_h1)
            phase2_half(1)

    nc.finalize()
    return nc


def _get_program():
    if "nc" not in _CACHE:
        _CACHE["nc"] = _build_program()
    return _CACHE["nc"]


def _host_prep(hidden_states, bias, cores):
    """Collapse TT cores to rank-5 factors; pack consts + x^T per core."""
    c0, c1, c2, c3, c4, c5 = [c.astype(np.float64) for c in cores]
    A = np.einsum("iv,vjw,wkx->ijkx", c0[0], c1, c2).reshape(HID, RANK)
    Bm = np.einsum("xpy,yqz,zr->xpqr", c3, c4, c5[:, :, 0]).reshape(RANK, HID)

    a_p = np.ascontiguousarray(
        A.reshape(FCH, 128, RANK).transpose(1, 0, 2).reshape(128, FCH * RANK)
    ).astype(ml_dtypes.bfloat16)                   # (128, 30)

    xts = []
    for c in range(NCORES):
        xc = hidden_states[c]  # (512, 768)
        xct = xc.T.astype(ml_dtypes.bfloat16)  # (768, 512)
        # per half: [p, c*sz + m~] = x^T[c*128+p, off+m~]
        blocks = [a_p]
        for h in (0, 1):
            sz, off = HSIZE[h], HOFF[h]
            blocks.append(
                xct[:, off : off + sz]
                .reshape(FCH, 128, sz)
                .transpose(1, 0, 2)
                .reshape(128, FCH * sz)
            )
        bm_blk = np.zeros((128, HID), dtype=ml_dtypes.bfloat16)
        bm_blk[:RANK] = Bm.astype(ml_dtypes.bfloat16)
        bm_blk[32] = bias.astype(ml_dtypes.bfloat16)
        blocks.append(bm_blk)
        xts.append(np.ascontiguousarray(np.concatenate(blocks, axis=1)))
    return xts


def _unpack_out(outt_list):
    """outt[p, h*1536 + j*256 + m~] = out[h*256+m~, j*128+p] -> (8,512,768)."""
    outs = []
    for outt in outt_list:
        o = (
            outt.reshape(128, 2, FCH, HSIZE[0])
            .transpose(1, 3, 2, 0)
            .reshape(ROWS, HID)
        )
        outs.append(o)
    return np.stack(outs, axis=0).astype(np.float32)


def run(inputs, trace=False, **spmd_kwargs):
    hidden_states = np.asarray(inputs["hidden_states"], dtype=np.float32)
    bias = np.asarray(inputs["bias"], dtype=np.float32)
    cores = [np.asarray(inputs[f"core{i}"], dtype=np.float32) for i in range(6)]

    xts = _host_prep(hidden_states, bias, cores)
    nc = _get_program()
    in_maps = [{"xt": xts[c]} for c in range(NCORES)]
    res = run_bass_kernel_spmd(
        nc, in_maps, core_ids=list(range(NCORES)), trace=trace, **spmd_kwargs
    )
    out = _unpack_out([res.results[c]["outt"] for c in range(NCORES)])
    if trace:
        return out, res
    return out


def kernel(**inputs):
    return run(inputs)
